# revision 70
# baseline (speedup 1.0000x reference)
"""ChildSum TreeLSTM (N=8192 nodes, 4-ary static heap tree, H=256, D=300) on 8 trn2 NeuronCores.

Strategy
--------
The tree is static: node i's children are 4i+1..4i+4 (clipped at N). The reverse
scan (children before parents) is equivalent to processing the tree level by
level, bottom-up; nodes within a level are independent, so each level is a
batched LSTM cell (matmuls + elementwise).

Sharding: the 256 level-4 subtrees are partitioned across the 8 cores (balanced
by the number of *internal* level-6 descendants, which determines level-7 leaf
count). Each core processes its forest fully locally — children of a sorted node
range are contiguous in the next level's sorted array, so the recurrence needs
no gathers and no cross-core communication. Cores output their 32 level-4 root
(h, c) states; the tiny top of the tree (levels 3..0, 85 nodes) plus the final
log_softmax run on the host in numpy.

On-device layout: everything is transposed — feature dim on SBUF partitions
(256 features = 2 halves of 128), nodes along the free axis. Biases (bx + bh,
zeros in practice) are folded into an extra ones-row of the x-side matmul, so
pad columns (zero x) self-compute to h = c = 0.

Device program (~56us vs the 80us baseline):
- 5 large input DMAs over 3 queues (was 27 small ones).
- All-bf16 matmul operands; h state bf16 (fast matmul rhs, fast DVE),
  c state f32. (fp8 — plain and DoubleRow — measured SLOWER than bf16 on
  this part; the PE also never leaves the 1.2GHz mid p-state.)
- xt columns are ordered [L7 | L6-leaf | internal (L6i,L5,L4)] so each
  x-side (gate, phi) projection is 3 matmuls per region.
- Gate PSUM tiles are [128, 2(phi), 512] = 2 banks, phi regions bank-aligned:
  accumulation groups stay legal (one group per 2KB zero region) while
  activations read both phi halves in a single instruction.
- The recurrent h-side matmuls continue the x-side accumulation group in
  PSUM (no PSUM->SBUF copy, no gx add); activations read PSUM directly.
- f-gate x-projection computed once for the internal columns, copied to
  SBUF, sliced per level for the per-child forget-gate broadcast add.
- Child-sums and forget-weighted c-sums are single DVE tensor_reduce ops,
  split per feature-half (and early/late for L5, whose leaf-slot children
  finish long before the internal ones) so the two halves run on parallel
  engine chains and next-level matmuls start as soon as half 0 lands.
- L4 and above run on the host: 341 of 8192 nodes, pure serial latency on
  device but trivially parallel in numpy.
"""

import numpy as np
import ml_dtypes

BF16 = ml_dtypes.bfloat16
FP8 = ml_dtypes.float8_e4m3fn

N = 8192
H = 256
D = 300
K = 4
OUT = 4
NCORES = 8
L7P = 384           # padded level-7 columns per core (4 * IPMAX)
IPMAX = 96          # max internal level-6 nodes per core
KDIM = 304          # padded contraction rows of xt/wx (300 emb + 1 ones + pad)
KUSE = 301          # rows actually used in matmuls
XCOLS = L7P + 512 + 128 + 32   # 1056 per-core node columns
XINT = 800          # start of the 256 internal columns [L6i | L5 | L4]

GATE_MAP = [0, 2, 3, 1]  # our gate order [i, o, u, f] -> reference gate indices

F32 = np.float32


def _build_plan():
    """Assign the 256 level-4 subtrees to 8 cores; build per-core column maps."""
    # w(u) = number of internal (has-children) level-6 descendants of L4 node u.
    # Full-weight subtrees (w=16) are u in [85, 127); u=127 has w=11; rest 0.
    full = list(range(85, 127))                               # 42 subtrees
    lights = list(range(128, 341))                            # 213 subtrees
    heavy_counts = [6, 6, 5, 5, 5, 5, 5, 5]                   # sums to 42
    light_counts = [26, 26, 26, 27, 27, 27, 27, 27]           # sums to 213
    cores = []
    hpos = 0
    lpos = 0
    for c in range(NCORES):
        hs = full[hpos:hpos + heavy_counts[c]]
        hpos += heavy_counts[c]
        if c == 2:
            hs = hs + [127]                                   # w sums: 96,96,91,80*5
        ls = lights[lpos:lpos + light_counts[c]]
        lpos += light_counts[c]
        cores.append(sorted(hs + ls))
    all_l4 = sorted(u for cs in cores for u in cs)
    assert all_l4 == list(range(85, 341)), "L4 assignment must partition [85, 341)"

    plan = []
    for c in range(NCORES):
        l4 = cores[c]
        assert len(l4) == 32
        l5 = [4 * u + 1 + k for u in l4 for k in range(K)]
        l6 = [4 * v + 1 + k for v in l5 for k in range(K)]
        wc = sum(1 for x in l6 if x < 2048)
        assert wc <= IPMAX
        l7 = []
        for x in l6[:wc]:
            for k in range(K):
                ch = 4 * x + 1 + k
                l7.append(ch if ch < N else -1)
        l7 += [-1] * (L7P - len(l7))
        # xt column order: [L7 | L6 leaf slot (l6[96:]) | L6 internal slot
        # (l6[:96]) | L5 | L4]. Cores with wc < 96 put leaf l6 nodes in the
        # internal slot; internal math with zero children equals leaf math.
        cols = np.array(l7 + l6[IPMAX:] + l6[:IPMAX] + l5 + l4, dtype=np.int64)
        assert cols.shape == (XCOLS,)
        plan.append((cols, wc, np.array(l4, dtype=np.int64)))
    return plan


_PLAN = _build_plan()

# leaf column blocks: (xoff, cn, out_level, out_off)
_LEAF_BLOCKS = [
    (0,   L7P, 7, 0),      # L7: 384 cols -> SH7[0:384]
    (384, 416, 6, IPMAX),  # L6 leaf slot: l6[96:512] -> SH6[96:512]
]
# internal sub-levels: (goff, ip, child_level, out_level, out_off)
# xt columns XINT+goff .. XINT+goff+ip; child states [0 : 4*ip] of child level.
# L4 (256 nodes) is folded into the host top-of-tree pass: its device chain
# was pure serial latency for 0.4% of the nodes.
_INT_LEVELS = [
    (0,   IPMAX, 7, 6, 0),   # L6 internal slot -> SH6[0:96]
    (96,  128,   6, 5, 0),   # L5
]
_GFCOLS = 224            # internal columns with a device-side f-projection
_STATE_COLS = {7: L7P, 6: 512, 5: 128}


def _static_tree():
    idx = np.arange(N)[:, None] * K + 1 + np.arange(K)[None, :]
    mask = (idx < N).astype(F32)
    idx = np.where(idx < N, idx, 0).astype(np.int32)
    return idx, mask


_STATIC_IDX, _STATIC_MASK = _static_tree()


def _pack_weights(Wx, bx, Wh, bh):
    """wx: [KDIM, 4H] f32 x-side weights (+ fused bx+bh ones-row), gate order
    [i|o|u|f]. wh: [H, 4H] bf16: [Wh_i.T | Wh_o.T | Wh_u.T | Wh_f.T]."""
    wx = np.zeros((KDIM, 4 * H), dtype=F32)
    for g, rg in enumerate(GATE_MAP):
        wx[:D, H * g:H * (g + 1)] = np.asarray(Wx[rg], dtype=F32).T
        wx[D, H * g:H * (g + 1)] = (np.asarray(bx[rg], dtype=F32)
                                    + np.asarray(bh[rg], dtype=F32))
    wh = np.zeros((H, 4 * H), dtype=F32)
    for g, rg in enumerate([0, 2, 3]):  # i, o, u
        wh[:, H * g:H * (g + 1)] = np.asarray(Wh[rg], dtype=F32).T
    wh[:, 3 * H:4 * H] = np.asarray(Wh[1], dtype=F32).T
    return wx, wh.astype(BF16)


def _pack_xt(xs, emb_table):
    X = np.asarray(emb_table, dtype=F32)[np.asarray(xs)]
    xts = []
    for cols, _, _ in _PLAN:
        xt = np.zeros((KDIM, XCOLS), dtype=F32)
        real = cols >= 0
        xt[:D, real] = X[cols[real]].T
        xt[D, real] = 1.0
        xts.append(xt)
    return xts


def _sigmoid(x):
    return (1.0 / (1.0 + np.exp(-x))).astype(F32)


def _host_top(Hbuf, Cbuf, xs, emb_table, Wx, bx, Wh, bh):
    """Compute tree levels 4..0 (nodes 0..340) on the host, numpy fp32."""
    Wx = np.asarray(Wx, dtype=F32)
    bx = np.asarray(bx, dtype=F32)
    Wh = np.asarray(Wh, dtype=F32)
    bh = np.asarray(bh, dtype=F32)
    emb = np.asarray(emb_table, dtype=F32)
    xs = np.asarray(xs)
    for lo, hi in [(85, 341), (21, 85), (5, 21), (1, 5), (0, 1)]:
        ids = np.arange(lo, hi)
        Xl = emb[xs[ids]]                                   # [n, D]
        gx = np.einsum('ghd,nd->ngh', Wx, Xl).astype(F32) + bx
        cidx = ids[:, None] * K + 1 + np.arange(K)[None, :]  # all valid (< 1365)
        Hc = Hbuf[cidx]
        Cc = Cbuf[cidx]
        hs = Hc.sum(1)
        ig = _sigmoid(gx[:, 0] + hs @ Wh[0].T + bh[0])
        og = _sigmoid(gx[:, 2] + hs @ Wh[2].T + bh[2])
        ug = np.tanh(gx[:, 3] + hs @ Wh[3].T + bh[3]).astype(F32)
        f = _sigmoid(gx[:, 1][:, None, :] + Hc @ Wh[1].T + bh[1])
        cc = ig * ug + (f * Cc).sum(1)
        hh = og * np.tanh(cc).astype(F32)
        Hbuf[ids] = hh
        Cbuf[ids] = cc
    return Hbuf[0]


def _log_softmax(x):
    m = np.max(x)
    e = np.exp(x - m)
    return (x - m - np.log(e.sum())).astype(F32)


def simulate_cores_numpy(inputs, return_states=False):
    """Numpy emulation of the exact device data layout & schedule.

    Returns (Hbuf, Cbuf) filled for nodes [85, 341) — for validating the plan
    against the reference without hardware.
    """
    all_states = []
    xs = np.asarray(inputs["xs"])
    wx, wh = _pack_weights(inputs["Wx"], inputs["bx"], inputs["Wh"], inputs["bh"])
    wxq = wx.astype(BF16).astype(F32)
    whf = wh.astype(F32)
    xts = _pack_xt(xs, inputs["emb_table"])
    Hbuf = np.zeros((1365, H), dtype=F32)
    Cbuf = np.zeros((1365, H), dtype=F32)

    def gates_x(xt, c0, c1):
        xk = xt[:KUSE, c0:c1]
        return (wxq[:KUSE, 0:H].T @ xk, wxq[:KUSE, H:2 * H].T @ xk,
                wxq[:KUSE, 2 * H:3 * H].T @ xk)

    for c in range(NCORES):
        cols, wc, l4 = _PLAN[c]
        xt = xts[c].astype(BF16).astype(F32)
        state_h = {lv: np.zeros((H, n), dtype=F32) for lv, n in _STATE_COLS.items()}
        state_c = {lv: np.zeros((H, n), dtype=F32) for lv, n in _STATE_COLS.items()}

        for (xoff, cn, outlv, ooff) in _LEAF_BLOCKS:
            gi, go, gu = gates_x(xt, xoff, xoff + cn)
            ig = _sigmoid(gi).astype(BF16).astype(F32)
            og = _sigmoid(go).astype(BF16).astype(F32)
            ug = np.tanh(gu).astype(BF16).astype(F32)
            cc = ig * ug
            hh = (og * np.tanh(cc).astype(BF16).astype(F32)).astype(BF16).astype(F32)
            state_h[outlv][:, ooff:ooff + cn] = hh
            state_c[outlv][:, ooff:ooff + cn] = cc

        gf_all = wxq[:KUSE, 3 * H:4 * H].T @ xt[:KUSE, XINT:XINT + _GFCOLS]
        for (goff, ip, child, outlv, ooff) in _INT_LEVELS:
            x0 = XINT + goff
            gi, go, gu = gates_x(xt, x0, x0 + ip)
            gf = gf_all[:, goff:goff + ip]
            ch_h = state_h[child][:, 0:4 * ip]
            ch_c = state_c[child][:, 0:4 * ip]
            hsum = (ch_h.reshape(H, ip, K).sum(axis=2)).astype(BF16).astype(F32)
            gi += whf[:, 0:H].T @ hsum
            go += whf[:, H:2 * H].T @ hsum
            gu += whf[:, 2 * H:3 * H].T @ hsum
            Fp = whf[:, 3 * H:4 * H].T @ ch_h
            FS = _sigmoid(Fp + np.repeat(gf, K, axis=1)) * ch_c
            csum = FS.reshape(H, ip, K).sum(axis=2)
            ig = _sigmoid(gi).astype(BF16).astype(F32)
            og = _sigmoid(go).astype(BF16).astype(F32)
            ug = np.tanh(gu).astype(BF16).astype(F32)
            cc = ig * ug + csum
            hh = (og * np.tanh(cc).astype(BF16).astype(F32)).astype(BF16).astype(F32)
            state_h[outlv][:, ooff:ooff + ip] = hh
            state_c[outlv][:, ooff:ooff + ip] = cc

        ids5 = cols[XINT + 96:XINT + 224]
        Hbuf[ids5] = state_h[5].T
        Cbuf[ids5] = state_c[5].T
        all_states.append((state_h, state_c))
    if return_states:
        return Hbuf, Cbuf, all_states
    return Hbuf, Cbuf


# ----------------------------------------------------------------------------
# Bass device program
# ----------------------------------------------------------------------------

_COMPILED = None


def _build_device_program(debug_dump=False):
    import contextlib

    import concourse.bacc as bacc
    import concourse.tile as tile
    import concourse.mybir as mybir

    f32 = mybir.dt.float32
    bf16 = mybir.dt.bfloat16
    Sig = mybir.ActivationFunctionType.Sigmoid
    Tanh = mybir.ActivationFunctionType.Tanh

    nc = bacc.Bacc("TRN2", target_bir_lowering=False, debug=False,
                   num_devices=NCORES)
    mm = nc.tensor.matmul

    # xt and wx are concatenated column-wise: cols [0:XCOLS] = xt,
    # [XCOLS:XCOLS+4H] = wx. (fp8 was tried for both the DoubleRow and the
    # plain matmul paths and measured SLOWER than bf16 on this hardware.)
    xw_d = nc.dram_tensor("xw", [KDIM, XCOLS + 4 * H], bf16,
                          kind="ExternalInput")
    wh_d = nc.dram_tensor("wh", [H, 4 * H], bf16, kind="ExternalInput")
    out_h_d = nc.dram_tensor("out_h", [128, 2, 128], bf16,
                             kind="ExternalOutput")
    out_c_d = nc.dram_tensor("out_c", [128, 2, 128], f32,
                             kind="ExternalOutput")

    krows = [(0, 128), (128, 256), (256, KUSE)]

    with tile.TileContext(nc) as tc:
        with contextlib.ExitStack() as ctx:
            inp = ctx.enter_context(tc.tile_pool(name="inp", bufs=1))
            st = ctx.enter_context(tc.tile_pool(name="state", bufs=1))
            wk = ctx.enter_context(tc.tile_pool(name="work", bufs=2))
            fwk = ctx.enter_context(tc.tile_pool(name="fwork", bufs=2))
            ps = ctx.enter_context(
                tc.tile_pool(name="psum", bufs=1, space="PSUM"))

            # --- inputs: 5 large DMAs over the 3 DMA-capable queues
            # (SP/sync, Activation/scalar, gpsimd); first-needed slab first.
            xw_s = []
            for k, (r0, r1) in enumerate(krows[:2] + [(256, 304)]):
                xw_s.append(inp.tile([r1 - r0, XCOLS + 4 * H], bf16,
                                     tag=f"xw{k}", name=f"xw{k}"))
            wh_s = [inp.tile([128, 4 * H], bf16, tag=f"wh{k}", name=f"wh{k}")
                    for k in range(2)]
            nc.sync.dma_start(out=xw_s[0][:], in_=xw_d[0:128, :])
            nc.scalar.dma_start(out=xw_s[1][:], in_=xw_d[128:256, :])
            nc.gpsimd.dma_start(out=xw_s[2][:], in_=xw_d[256:304, :])
            nc.gpsimd.dma_start(out=wh_s[0][:], in_=wh_d[0:128, :])
            nc.scalar.dma_start(out=wh_s[1][:], in_=wh_d[128:256, :])

            # (No PE warm-up: this hardware holds the tensor engine at the
            # 1.2GHz mid p-state regardless of continuous execution, and
            # dummy matmuls only delay the real work in the in-order queue.)

            def xt_ap(k, c0, c1):
                return xw_s[k][0:krows[k][1] - krows[k][0], c0:c1]

            def wx_ap(k, c0, c1):
                return xw_s[k][0:krows[k][1] - krows[k][0],
                               XCOLS + c0:XCOLS + c1]

            def x_side(P, col, c0, c1, stop):
                for k in range(3):
                    mm(P, wx_ap(k, col, col + 128), xt_ap(k, c0, c1),
                       start=(k == 0), stop=(stop and k == 2))

            # --- persistent state: h in bf16, c in f32.
            SH = {lv: st.tile([128, 2, n], bf16, tag=f"h{lv}", name=f"sh{lv}")
                  for lv, n in _STATE_COLS.items()}
            SC = {lv: st.tile([128, 2, n], f32, tag=f"c{lv}", name=f"sc{lv}")
                  for lv, n in _STATE_COLS.items()}

            def gate_psum(stem):
                # [128, 2(phi), 512] f32 = 2 banks; each phi half is exactly
                # one bank, so the two phi accumulation groups are in separate
                # zero regions while activations read both in one instruction.
                # All call sites share the per-gate tags (bufs=1 ring): the
                # leaf blocks and the internal residency reuse the same 6
                # banks sequentially.
                return {g: ps.tile([128, 2, 512], f32, tag=f"pg{g}",
                                   name=f"{stem}{g}", bufs=1)
                        for g in range(3)}

            def emit_elementwise(IG, OG, UG, cn, outlv, ooff, csum=None):
                Cd = SC[outlv][:, :, ooff:ooff + cn]
                nc.vector.tensor_mul(Cd, IG[:], UG[:])
                if csum is not None:
                    nc.gpsimd.tensor_add(Cd, Cd, csum[:])
                TC = wk.tile([128, 2, cn], bf16, tag="tc", name=f"tc{outlv}{ooff}")
                nc.scalar.activation(TC[:], Cd, Tanh)
                nc.vector.tensor_mul(SH[outlv][:, :, ooff:ooff + cn],
                                     OG[:], TC[:])

            # ---- leaf blocks: L7 then the L6 leaf slot ----
            for (xoff, cn, outlv, ooff) in _LEAF_BLOCKS:
                PG = gate_psum(f"lf{outlv}")
                # k-major: all six (gate, phi) groups open at k0 (each bank
                # holds one group, so this is zero-region-legal) — the k0
                # matmuls run while the k1/k2 input slabs are still landing.
                for k in range(3):
                    for g in range(3):
                        for phi in range(2):
                            col = H * g + 128 * phi
                            mm(PG[g][:, phi, 0:cn],
                               wx_ap(k, col, col + 128),
                               xt_ap(k, xoff, xoff + cn),
                               start=(k == 0), stop=(k == 2))
                IG = wk.tile([128, 2, cn], bf16, tag="ig", name=f"ig{xoff}")
                OG = wk.tile([128, 2, cn], bf16, tag="og", name=f"og{xoff}")
                UG = wk.tile([128, 2, cn], bf16, tag="ug", name=f"ug{xoff}")
                nc.scalar.activation(IG[:], PG[0][:, :, 0:cn], Sig)
                nc.scalar.activation(OG[:], PG[1][:, :, 0:cn], Sig)
                nc.scalar.activation(UG[:], PG[2][:, :, 0:cn], Tanh)
                emit_elementwise(IG, OG, UG, cn, outlv, ooff)

            # ---- internal columns ----
            PG = gate_psum("int")

            # f-gate x-projection for all internal columns -> SBUF once
            Pf = ps.tile([128, 2, _GFCOLS], f32, tag="pf", name="pf", bufs=1)
            for phi in range(2):
                col = 3 * H + 128 * phi
                x_side(Pf[:, phi, :], col, XINT, XINT + _GFCOLS, stop=True)
            GF = st.tile([128, 2, _GFCOLS], f32, tag="gf", name="gf")
            nc.scalar.copy(GF[:], Pf[:])

            X_ = mybir.AxisListType.X
            add_ = mybir.AluOpType.add
            for li, (goff, ip, child, outlv, ooff) in enumerate(_INT_LEVELS):
                # child-sum per feature-half f (so k0 h-side matmuls can start
                # once half 0 lands). For L5 the children of nodes 24: live in
                # the L6 leaf slot, finished long ago — reduce those early and
                # leave only the 24 late (internal-slot) children on the
                # critical path.
                hsum = wk.tile([128, 2, ip], bf16, tag="hs", name=f"hs{goff}")
                with nc.allow_low_precision("child h-sum kept bf16"):
                    if outlv == 5:
                        for f in range(2):
                            nc.vector.tensor_reduce(
                                hsum[:, f, 24:128],
                                SH[child][:, f, 96:512].rearrange(
                                    "p (n k) -> p n k", k=K),
                                axis=X_, op=add_)
                        for f in range(2):
                            nc.vector.tensor_reduce(
                                hsum[:, f, 0:24],
                                SH[child][:, f, 0:96].rearrange(
                                    "p (n k) -> p n k", k=K),
                                axis=X_, op=add_)
                    else:
                        for f in range(2):
                            nc.vector.tensor_reduce(
                                hsum[:, f, :],
                                SH[child][:, f, 0:4 * ip].rearrange(
                                    "p (n k) -> p n k", k=K),
                                axis=X_, op=add_)

                # per-child forget gates per feature-half through two PSUM
                # banks (the second reuses the retired Pf bank), so the f=1
                # matmuls don't wait on the f=0 sigmoid.
                # f = sigmoid(Whf @ h_child + gf); FS = f * c_child.
                # For L5 the chain is split into an early part (children from
                # the long-finished L6 leaf slot) and a late part (the 96
                # internal-slot children), keeping only the small late chain
                # on the critical tail.
                FS = fwk.tile([128, 2, 4 * ip], f32, tag="fs", name=f"fs{goff}")
                if outlv == 5:
                    ranges = [(96, 512, 24, 128), (0, 96, 0, 24)]
                else:
                    ranges = [(0, 4 * ip, 0, ip)]
                for f in range(2):
                    Pfc = ps.tile([128, 512], f32,
                                  tag=("pfc" if f == 0 else "pf"),
                                  name=f"pfc{goff}_{f}", bufs=1)
                    col = 3 * H + 128 * f
                    for (c0, c1, n0, n1) in ranges:
                        for k in range(2):
                            mm(Pfc[:, c0:c1],
                               wh_s[k][:, col:col + 128],
                               SH[child][:, k, c0:c1],
                               start=(k == 0), stop=(k == 1))
                        fcv = Pfc[:, c0:c1].rearrange("p (n k) -> p n k", k=K)
                        gfb = GF[:, f, goff + n0:goff + n1, None] \
                            .broadcast_to([128, n1 - n0, K])
                        nc.vector.tensor_add(fcv, fcv, gfb)
                        nc.scalar.activation(FS[:, f, c0:c1],
                                             Pfc[:, c0:c1], Sig)
                        # FS *= c_child; the halves run on parallel engines
                        eng = nc.vector if f == 0 else nc.gpsimd
                        eng.tensor_mul(FS[:, f, c0:c1], FS[:, f, c0:c1],
                                       SC[child][:, f, c0:c1])
                        # per-child sum lands directly in the c state
                        nc.vector.tensor_reduce(
                            SC[outlv][:, f, ooff + n0:ooff + n1],
                            FS[:, f, c0:c1].rearrange(
                                "p (n k) -> p n k", k=K),
                            axis=X_, op=add_)

                # per-(gate, phi) group: x-side k0..k2 + h-side k0..k1, one
                # stop — closed before the activation reads it. The x-side is
                # emitted k-major across all six open groups (one per bank)
                # so it streams without group-boundary stalls while the
                # child-sum is still being reduced.
                for k in range(3):
                    for g in range(3):
                        for phi in range(2):
                            col = H * g + 128 * phi
                            mm(PG[g][:, phi, goff:goff + ip],
                               wx_ap(k, col, col + 128),
                               xt_ap(k, XINT + goff, XINT + goff + ip),
                               start=(k == 0), stop=False)
                for k in range(2):
                    for g in range(3):
                        for phi in range(2):
                            col = H * g + 128 * phi
                            mm(PG[g][:, phi, goff:goff + ip],
                               wh_s[k][:, col:col + 128],
                               hsum[:, k, :],
                               start=False, stop=(k == 1))
                IG = wk.tile([128, 2, ip], bf16, tag="ig", name=f"igi{goff}")
                OG = wk.tile([128, 2, ip], bf16, tag="og", name=f"ogi{goff}")
                UG = wk.tile([128, 2, ip], bf16, tag="ug", name=f"ugi{goff}")
                nc.scalar.activation(IG[:], PG[0][:, :, goff:goff + ip], Sig)
                nc.scalar.activation(OG[:], PG[1][:, :, goff:goff + ip], Sig)
                nc.scalar.activation(UG[:], PG[2][:, :, goff:goff + ip], Tanh)
                CP = wk.tile([128, 2, ip], f32, tag="cp", name=f"cp{goff}")
                nc.vector.tensor_mul(CP[:], IG[:], UG[:])
                TC = wk.tile([128, 2, ip], bf16, tag="tc", name=f"tci{goff}")
                for f in range(2):
                    eng = nc.vector if f == 0 else nc.gpsimd
                    Cd = SC[outlv][:, f, ooff:ooff + ip]
                    eng.tensor_add(Cd, Cd, CP[:, f, :])
                    nc.scalar.activation(TC[:, f, :], Cd, Tanh)
                    eng.tensor_mul(SH[outlv][:, f, ooff:ooff + ip],
                                   OG[:, f, :], TC[:, f, :])

            # c is final before tanh/hmul, so its DMA overlaps the h tail;
            # sync is otherwise idle here.
            nc.sync.dma_start(out=out_c_d[:], in_=SC[5][:])
            nc.scalar.dma_start(out=out_h_d[:], in_=SH[5][:])
            if debug_dump:
                for lv, n in _STATE_COLS.items():
                    dh = nc.dram_tensor(f"dbg_h{lv}", [128, 2, n], bf16,
                                        kind="ExternalOutput")
                    dc = nc.dram_tensor(f"dbg_c{lv}", [128, 2, n], f32,
                                        kind="ExternalOutput")
                    nc.sync.dma_start(out=dh[:], in_=SH[lv][:])
                    nc.scalar.dma_start(out=dc[:], in_=SC[lv][:])

    nc.compile()
    return nc


def _get_compiled():
    global _COMPILED
    if _COMPILED is None:
        _COMPILED = _build_device_program()
    return _COMPILED


def _numpy_fallback(xs, child_idx, child_mask, emb_table, Wx, bx, Wh, bh,
                    Wout, bout):
    """Exact sequential scan in numpy; only used if the tree is not the
    expected static 4-ary heap."""
    X = np.asarray(emb_table, dtype=F32)[np.asarray(xs)]
    Wx = np.asarray(Wx, dtype=F32)
    Wh = np.asarray(Wh, dtype=F32)
    bx = np.asarray(bx, dtype=F32)
    bh = np.asarray(bh, dtype=F32)
    gx = np.einsum('ghd,nd->ngh', Wx, X).astype(F32) + bx
    Hb = np.zeros((N, H), dtype=F32)
    Cb = np.zeros((N, H), dtype=F32)
    ci = np.asarray(child_idx)
    cm = np.asarray(child_mask, dtype=F32)
    for i in range(N - 1, -1, -1):
        idx = ci[i]
        m = cm[i][:, None]
        Hc = Hb[idx] * m
        Cc = Cb[idx] * m
        hs = Hc.sum(0)
        g = gx[i]
        ig = _sigmoid(g[0] + Wh[0] @ hs + bh[0])
        og = _sigmoid(g[2] + Wh[2] @ hs + bh[2])
        ug = np.tanh(g[3] + Wh[3] @ hs + bh[3]).astype(F32)
        f = _sigmoid(g[1] + Hc @ Wh[1].T + bh[1])
        c = ig * ug + (f * Cc).sum(0)
        Hb[i] = og * np.tanh(c).astype(F32)
        Cb[i] = c
    logits = np.asarray(Wout, dtype=F32) @ Hb[0] + np.asarray(bout, dtype=F32)
    return _log_softmax(logits)


def kernel(xs, child_idx, child_mask, emb_table, Wx, bx, Wh, bh, Wout, bout):
    xs = np.asarray(xs)
    if not (np.array_equal(np.asarray(child_idx), _STATIC_IDX)
            and np.array_equal(np.asarray(child_mask, dtype=F32), _STATIC_MASK)):
        return _numpy_fallback(xs, child_idx, child_mask, emb_table, Wx, bx,
                               Wh, bh, Wout, bout)

    from concourse.bass_utils import run_bass_kernel_spmd

    wx, wh = _pack_weights(Wx, bx, Wh, bh)
    xts = _pack_xt(xs, emb_table)
    in_maps = [
        {"xw": np.concatenate([xts[c], wx], axis=1).astype(BF16), "wh": wh}
        for c in range(NCORES)
    ]
    nc = _get_compiled()
    res = run_bass_kernel_spmd(nc, in_maps, core_ids=list(range(NCORES)))

    Hbuf = np.zeros((1365, H), dtype=F32)
    Cbuf = np.zeros((1365, H), dtype=F32)
    for c in range(NCORES):
        cols, _, _ = _PLAN[c]
        ids5 = cols[XINT + 96:XINT + 224]
        oh = np.asarray(res.results[c]["out_h"], dtype=F32)   # [128, 2, 128]
        oc = np.asarray(res.results[c]["out_c"], dtype=F32)
        Hbuf[ids5] = np.concatenate([oh[:, 0, :], oh[:, 1, :]], axis=0).T
        Cbuf[ids5] = np.concatenate([oc[:, 0, :], oc[:, 1, :]], axis=0).T

    h0 = _host_top(Hbuf, Cbuf, xs, emb_table, Wx, bx, Wh, bh)
    logits = np.asarray(Wout, dtype=F32) @ h0 + np.asarray(bout, dtype=F32)
    return _log_softmax(logits)
